# revision 27
# baseline (speedup 1.0000x reference)
"""Trainium2 Bass kernel for the 4-qubit variational-circuit batch evaluator.

Math: the circuit output is exactly out[b, w] = sum_m C[w, m] * F_m(x_b)
where F_m are the 81 products of per-wire features {1, cos x_v, sin x_v}
(Pauli strings with X vanish on the RX product state).  C depends only on
`weights` and is recovered on the host (f64 lstsq against a tiny numpy
re-implementation of the 16-dim circuit).  Terms are pruned by magnitude
(cumulative |C| drop budget 1e-2 — rel err ~2.9e-3, well under the 2e-2
tolerance).  For the nominal small-weight regime only 4 terms survive:
out0=cos x0, out1=cos x1, out2=cos x0*cos x2, out3=cos x1*cos x3 (all
coefficients within 4e-4 of 1, so they are dropped too).

Fast-path device kernel per core (data parallel, batch sharded 8 ways;
8 chunks, each one contiguous 256 KB DRAM block, fully linear DMAs):
  - x shard [131072, 4] viewed as [128 partitions, 1024 samples, 4 ch],
    tiles interleaved [p, n, 4ch]
  - DVE add_range_wrap brings cos arguments into [-pi, pi] (whole tile)
  - ACT Sin writes cos of all 4 channels straight into the out tile
  - GPSIMD multiplies ch2 *= ch0 and ch3 *= ch1 in place
Each engine has exactly one role per chunk so the in-order pipelines can
never invert across chunks, and every engine stays well under the
1456 ns/chunk DMA cadence, keeping the (serialized, 360 GB/s) DMA engine
streaming the 16 x 256 KB transfers back to back.

The default FAST_MODE="raw" emits this hand-scheduled with explicit
semaphores and no TileContext/BassBlock at all, which drops the tile
framework's entry branches and scope-exit teardown (sem recycling + two
barrier rounds): ~1866 ns ramp (framework preamble + first-DMA
HWDGE/DGE latency) + 11648 ns DMA + 975 ns completion-sem tail
= 14489 ns — every remaining component is intrinsic (framework preamble,
DMA pipeline latency, completion-receipt propagation).

A generic balanced emitter (the previous baseline) remains as fallback
for weights whose surviving-term structure differs.
"""
import math
import sys

import numpy as np

sys.path.insert(0, "/opt/trn_rl_repo")

N_QUBITS = 4
N_LAYERS = 2
CNOT_PAIRS = [(i, j) for i in range(N_QUBITS) for j in range(i + 1, N_QUBITS)]
B_TOTAL = 1048576
N_CORES = 8
S_CORE = B_TOTAL // N_CORES        # 131072 samples per core
P = 128
NPP = S_CORE // P                  # 1024 samples per partition
NCHUNK = 8
NB = NPP // NCHUNK                 # samples per partition per chunk

DROP_BUDGET = 1e-2                 # max cumulative |C| pruned away
FAST_MODE = "raw"                  # fast-path variant: "raw" (hand-scheduled,
                                   # no tile framework) or "inplace" (TileContext)


# ---------------------------------------------------------------- host math
def _circuit_outputs(x, weights):
    """f64 numpy re-implementation of the reference circuit. [B,4] -> [B,4]."""
    B = x.shape[0]
    state = np.zeros((B,) + (2,) * N_QUBITS, dtype=np.complex128)
    state[(slice(None),) + (0,) * N_QUBITS] = 1.0

    def apply_1q(state, gate, wire):
        s = np.moveaxis(state, wire + 1, -1)
        if gate.ndim == 3:
            s = np.einsum("bij,b...j->b...i", gate, s)
        else:
            s = np.einsum("ij,b...j->b...i", gate, s)
        return np.moveaxis(s, -1, wire + 1)

    for w in range(N_QUBITS):
        th = x[:, w] * 0.5
        c = np.cos(th)
        s = -1j * np.sin(th)
        gate = np.stack([np.stack([c, s], -1), np.stack([s, c], -1)], -2)
        state = apply_1q(state, gate, w)
    for l in range(N_LAYERS):
        for (ctrl, tgt) in CNOT_PAIRS:
            s0 = np.take(state, 0, axis=ctrl + 1)
            s1 = np.take(state, 1, axis=ctrl + 1)
            s1 = np.flip(s1, axis=tgt)
            state = np.stack([s0, s1], axis=ctrl + 1)
        for w in range(N_QUBITS):
            a = weights[l, w] * 0.5
            gate = np.array(
                [[np.cos(a), -np.sin(a)], [np.sin(a), np.cos(a)]],
                dtype=np.complex128,
            )
            state = apply_1q(state, gate, w)
    probs = np.abs(state) ** 2
    outs = []
    for w in range(N_QUBITS):
        p = np.moveaxis(probs, w + 1, 1).reshape(B, 2, -1)
        outs.append(p[:, 0].sum(-1) - p[:, 1].sum(-1))
    return np.stack(outs, -1)


def _features(x):
    """Trig features, kron over wires of [1, cos, sin]. [B,4] -> [B,81]."""
    B = x.shape[0]
    F = np.ones((B, 1))
    for v in range(N_QUBITS):
        g = np.stack([np.ones(B), np.cos(x[:, v]), np.sin(x[:, v])], -1)
        F = (F[:, :, None] * g[:, None, :]).reshape(B, -1)
    return F


def _solve_C(weights):
    """[4, 81] coefficient matrix, exact up to f64 lstsq noise (~1e-13)."""
    rng = np.random.default_rng(1234)
    xs = rng.normal(size=(486, N_QUBITS))
    F = _features(xs)
    Y = _circuit_outputs(xs, weights)
    C, *_ = np.linalg.lstsq(F, Y, rcond=None)
    return C.T


def _select_terms(C):
    """Prune smallest coefficients with cumulative |C| <= DROP_BUDGET.

    Returns per-output lists of (coeff, factors) with factors a tuple of
    (wire, kind) and kind in {"c", "s"}.
    """
    flat = np.abs(C).ravel()
    order = np.argsort(flat)
    cum = np.cumsum(flat[order])
    drop = set(order[cum <= DROP_BUDGET].tolist())
    terms = [[] for _ in range(N_QUBITS)]
    for w in range(N_QUBITS):
        for m in range(81):
            if abs(C[w, m]) == 0.0 or (w * 81 + m) in drop:
                continue
            digits = [(m // 27) % 3, (m // 9) % 3, (m // 3) % 3, m % 3]
            factors = tuple(
                (v, "c" if d == 1 else "s")
                for v, d in enumerate(digits)
                if d != 0
            )
            terms[w].append((float(C[w, m]), factors))
    return terms


def _progression(chans):
    """Smallest arithmetic progression (offset, step, count) covering chans."""
    chans = sorted(set(chans))
    if len(chans) == 1:
        return chans[0], 1, 1
    diffs = [b - a for a, b in zip(chans, chans[1:])]
    step = diffs[0]
    for d in diffs[1:]:
        step = math.gcd(step, d)
    count = (chans[-1] - chans[0]) // step + 1
    return chans[0], step, count


# ------------------------------------------------------------ bass program
def _fast_structure(terms):
    """Detect the diagonal term structure produced by small RY weights:
    out0=a0*c0, out1=a1*c1, out2=a2*c0*c2, out3=a3*c1*c3.  Returns the four
    coefficients, or None if the structure doesn't match."""
    try:
        (a0, f0), = terms[0]
        (a1, f1), = terms[1]
        (a2, f2), = terms[2]
        (a3, f3), = terms[3]
    except (ValueError, IndexError):
        return None
    if (f0 == ((0, "c"),) and f1 == ((1, "c"),)
            and f2 == ((0, "c"), (2, "c")) and f3 == ((1, "c"), (3, "c"))
            and max(abs(1.0 - a) for a in (a0, a1, a2, a3)) < 1.5e-3):
        # the fast path drops the ~1 coefficients entirely; only valid
        # when that adds well under the tolerance
        return (a0, a1, a2, a3)
    return None


def _build_fast(coefs, reps=1, nchunk=NCHUNK, bufs=8, mode="inplace"):
    """Specialized 3-op-per-chunk pipeline for the diagonal structure.

    Per chunk (all tiles interleaved [p, n, 4ch], one contiguous 256 KB DMA
    each way):
      DVE : add_range_wrap over the whole tile  (cos args into [-pi,pi])
      ACT : Sin -> cos of all 4 channels
      POOL: out[ch0:2]  = mean(a0,a1) * cos[ch0:2]      (strided 1-in op)
      DVE : out[ch2:4]  = mean(a2,a3) * cos[ch0:2] * cos[ch2:4]  (strided stt)
    Products + out-DMA are emitted under high_priority so finished chunks
    drain immediately and the out-DMA stream tucks in right behind the
    8 in-DMAs on the (serialized) DMA engines.
    """
    import concourse.bacc as bacc
    import concourse.tile as tile
    from concourse import mybir

    f32 = mybir.dt.float32
    Sin = mybir.ActivationFunctionType.Sin
    mult = mybir.AluOpType.mult
    PI = float(np.pi)
    HALF_PI = float(np.pi / 2)
    nb = NPP // nchunk
    a0, a1, a2, a3 = coefs
    c01 = float((a0 + a1) / 2)
    c23 = float((a2 + a3) / 2)

    nc = bacc.Bacc("TRN2", target_bir_lowering=False, debug=False,
                   num_devices=N_CORES)
    x_d = nc.dram_tensor("x", [S_CORE, N_QUBITS], f32, kind="ExternalInput").ap()
    o_d = nc.dram_tensor("out", [S_CORE, N_QUBITS], f32,
                         kind="ExternalOutput").ap()
    x2 = x_d.rearrange("(k p n) c -> k p (n c)", k=nchunk, p=P)
    o2 = o_d.rearrange("(k p n) c -> k p (n c)", k=nchunk, p=P)

    with tile.TileContext(nc) as tc:
        with tc.tile_pool(name="xp", bufs=bufs) as xp, \
             tc.tile_pool(name="wp", bufs=bufs) as wp, \
             tc.tile_pool(name="cp", bufs=bufs) as cp, \
             tc.tile_pool(name="op", bufs=bufs) as op:
            for k in range(nchunk * reps):
                k = k % nchunk
                xt = xp.tile([P, 4 * nb], f32)
                nc.sync.dma_start(xt[:], x2[k])
                wt = wp.tile([P, 4 * nb], f32)
                nc.vector.add_range_wrap(wt[:], xt[:], shift=HALF_PI,
                                         bound=PI, period=2 * PI)
                # Each engine has a single role so the in-order pipelines
                # can never invert across chunks: DVE wraps, ACT evaluates
                # cos, Pool forms the ch2/ch3 products.  The ~1 coefficients
                # are dropped or folded; total approximation error ~3e-3
                # against the 2e-2 tolerance.
                ot = op.tile([P, 4 * nb], f32)
                orr = ot[:].rearrange("p (n c) -> p n c", c=4)
                if mode == "inplace":
                    # Sin writes cos of all 4 channels straight into the out
                    # tile; Pool multiplies ch2/ch3 by ch0/ch1 in place.
                    nc.scalar.activation(ot[:], wt[:], Sin)
                    with tc.high_priority(offset=100000):
                        nc.gpsimd.tensor_tensor(orr[:, :, 2], orr[:, :, 0],
                                                orr[:, :, 2], mult)
                        nc.gpsimd.tensor_tensor(orr[:, :, 3], orr[:, :, 1],
                                                orr[:, :, 3], mult)
                        nc.sync.dma_start(o2[k], ot[:])
                elif mode == "copies":
                    # Sin into a temp; ACT copies ch0/ch1 (exact coeffs),
                    # Pool products ch2/ch3 from the temp.
                    Copy = mybir.ActivationFunctionType.Copy
                    ct = cp.tile([P, 4 * nb], f32)
                    nc.scalar.activation(ct[:], wt[:], Sin)
                    cr = ct[:].rearrange("p (n c) -> p n c", c=4)
                    with tc.high_priority(offset=100000):
                        nc.scalar.activation(orr[:, :, 0], cr[:, :, 0],
                                             Copy, scale=float(a0))
                        nc.scalar.activation(orr[:, :, 1], cr[:, :, 1],
                                             Copy, scale=float(a1))
                        nc.gpsimd.tensor_tensor(orr[:, :, 2], cr[:, :, 0],
                                                cr[:, :, 2], mult)
                        nc.gpsimd.tensor_tensor(orr[:, :, 3], cr[:, :, 1],
                                                cr[:, :, 3], mult)
                        nc.sync.dma_start(o2[k], ot[:])
                else:  # "pool3"
                    ct = cp.tile([P, 4 * nb], f32)
                    nc.scalar.activation(ct[:], wt[:], Sin)
                    cr = ct[:].rearrange("p (n c) -> p n c", c=4)
                    with tc.high_priority(offset=100000):
                        nc.gpsimd.tensor_scalar(orr[:, :, 0:2], cr[:, :, 0:2],
                                                c01, None, mult)
                        nc.gpsimd.tensor_tensor(orr[:, :, 2], cr[:, :, 0],
                                                cr[:, :, 2], mult)
                        nc.gpsimd.tensor_tensor(orr[:, :, 3], cr[:, :, 1],
                                                cr[:, :, 3], mult)
                        nc.sync.dma_start(o2[k], ot[:])

    nc.compile()
    from concourse.bass_interp import get_hw_module
    nc.m = get_hw_module(nc.m)
    return nc


def _build_raw(coefs, reps=1, nchunk=NCHUNK):
    """Hand-scheduled variant of the fast path: no TileContext, manual
    semaphores.  Saves the tile framework's scope-exit teardown (sem
    recycling + an extra barrier round) at the program tail.

    Dependency protocol (r = rep, j = tile slot, i = r*nchunk+j):
      in-DMA  -> sem_in[j] += 16  (slot reuse guarded by sem_w >= i-nchunk+1)
      wrap    -> sem_w += 1       (waits sem_in[j] >= 16(r+1); sem_s for reuse)
      Sin     -> sem_s += 1       (waits sem_w >= i+1; sem_out[j] for reuse)
      2x Pool -> sem_p += 1 each  (waits sem_s >= i+1)
      out-DMA -> sem_out[j] += 16 (waits sem_p >= 2(i+1))
    DMA completion sems are PER SLOT: a DMA's +16 arrives as 16 interleaved
    per-engine increments, so a shared counter has no stable intermediate
    wait points when two DMAs are in flight (the tile framework rotates
    8 sem lanes for the same reason).  The slot-reuse guards ensure at most
    one in-flight DMA per slot sem.

    The program ends with SP waiting every out-DMA completion sem.  (An
    experiment without the final waits modeled 25 ns faster and produced
    bit-identical outputs across repeated executions, but a later process
    start hit NRT_EXEC_UNIT_UNRECOVERABLE — ending execution with DMAs
    possibly in flight risks wedging the device at unload.  Keep the waits.)
    """
    import concourse.bacc as bacc
    from concourse import mybir

    f32 = mybir.dt.float32
    Sin = mybir.ActivationFunctionType.Sin
    mult = mybir.AluOpType.mult
    PI = float(np.pi)
    HALF_PI = float(np.pi / 2)
    nb = NPP // nchunk
    n = nchunk * reps

    nc = bacc.Bacc("TRN2", target_bir_lowering=False, debug=False,
                   num_devices=N_CORES)
    x_d = nc.dram_tensor("x", [S_CORE, N_QUBITS], f32, kind="ExternalInput").ap()
    o_d = nc.dram_tensor("out", [S_CORE, N_QUBITS], f32,
                         kind="ExternalOutput").ap()
    x2 = x_d.rearrange("(k p n) c -> k p (n c)", k=nchunk, p=P)
    o2 = o_d.rearrange("(k p n) c -> k p (n c)", k=nchunk, p=P)

    xts = [nc.alloc_sbuf_tensor(f"xt{j}", [P, 4 * nb], f32)
           for j in range(nchunk)]
    wts = [nc.alloc_sbuf_tensor(f"wt{j}", [P, 4 * nb], f32)
           for j in range(nchunk)]
    ots = [nc.alloc_sbuf_tensor(f"ot{j}", [P, 4 * nb], f32)
           for j in range(nchunk)]
    sem_in = [nc.alloc_semaphore(f"sem_in{j}") for j in range(nchunk)]
    sem_out = [nc.alloc_semaphore(f"sem_out{j}") for j in range(nchunk)]
    sem_w = nc.alloc_semaphore("sem_w")
    sem_s = nc.alloc_semaphore("sem_s")
    sem_p = nc.alloc_semaphore("sem_p")

    # Direct emission into the entry block — no TileContext, no BassBlock.
    # The out-DMA sem waits on SP already guarantee completion, there is no
    # collective kernel-end inc for this program, and each engine's stream
    # may simply end: this drops both the block-entry branches and the
    # block-exit all-engine barrier (~450 ns total at ramp + tail).
    for r in range(reps):
        for j in range(nchunk):
            i = r * nchunk + j
            if i >= nchunk:
                nc.sync.wait_ge(sem_w, i - nchunk + 1)
            nc.sync.dma_start(xts[j][:], x2[j]).then_inc(sem_in[j], 16)
        for j in range(nchunk):
            i = r * nchunk + j
            nc.sync.wait_ge(sem_p, 2 * (i + 1))
            nc.sync.dma_start(o2[j], ots[j][:]).then_inc(sem_out[j], 16)
    for j in range(nchunk):
        nc.sync.wait_ge(sem_out[j], 16 * reps)

    for i in range(n):
        r, j = divmod(i, nchunk)
        nc.vector.wait_ge(sem_in[j], 16 * (r + 1))
        if i >= nchunk:
            nc.vector.wait_ge(sem_s, i - nchunk + 1)
        nc.vector.add_range_wrap(wts[j][:], xts[j][:], shift=HALF_PI,
                                 bound=PI, period=2 * PI).then_inc(sem_w, 1)

    for i in range(n):
        r, j = divmod(i, nchunk)
        nc.scalar.wait_ge(sem_w, i + 1)
        if i >= nchunk:
            nc.scalar.wait_ge(sem_out[j], 16 * r)
        nc.scalar.activation(ots[j][:], wts[j][:], Sin).then_inc(sem_s, 1)

    for i in range(n):
        j = i % nchunk
        nc.gpsimd.wait_ge(sem_s, i + 1)
        orr = ots[j][:].rearrange("p (n c) -> p n c", c=4)
        nc.gpsimd.tensor_tensor(orr[:, :, 2], orr[:, :, 0], orr[:, :, 2],
                                mult).then_inc(sem_p, 1)
        nc.gpsimd.tensor_tensor(orr[:, :, 3], orr[:, :, 1], orr[:, :, 3],
                                mult).then_inc(sem_p, 1)

    nc.compile()
    from concourse.bass_interp import get_hw_module
    nc.m = get_hw_module(nc.m)
    return nc


class _Balancer:
    """Greedy DVE/GPSIMD placement by modeled busy-ns (DVE 0.96 GHz 1x;
    GPSIMD ~line-rate 1-input, ~2x slower 2-input, 1.2 GHz)."""

    def __init__(self, nc, use_gpsimd, gp_two_in=True):
        self.nc = nc
        self.use_gpsimd = use_gpsimd
        self.gp_two_in = gp_two_in
        self.busy = {"v": 0.0, "g": 0.0}

    def _pick(self, cv, cg):
        if not self.use_gpsimd:
            self.busy["v"] += cv
            return self.nc.vector
        if self.busy["v"] + cv <= self.busy["g"] + cg:
            self.busy["v"] += cv
            return self.nc.vector
        self.busy["g"] += cg
        return self.nc.gpsimd

    def one_in(self, n):          # tensor_scalar / copy
        return self._pick((n + 110) / 0.96, (n + 250) / 1.2)

    def two_in(self, n, is_tt=False):  # tensor_tensor / scalar_tensor_tensor
        allow = self.gp_two_in is True or (self.gp_two_in == "tt" and is_tt)
        if not allow:
            self.busy["v"] += (n + 160) / 0.96
            return self.nc.vector
        return self._pick((n + 160) / 0.96, (2 * n + 250) / 1.2)

    def dve_only(self, n):        # custom ops (add_range_wrap)
        self.busy["v"] += (n + 160) / 0.96
        return self.nc.vector


def _build_program(terms, reps=1, use_gpsimd=True, nchunk=NCHUNK,
                   bufs=8, out_eng="sync", gp_two_in="tt"):
    coefs = _fast_structure(terms)
    if coefs is not None:
        if FAST_MODE == "raw":
            return _build_raw(coefs, reps=reps, nchunk=nchunk)
        return _build_fast(coefs, reps=reps, nchunk=nchunk, bufs=bufs,
                           mode=FAST_MODE)
    # off-nominal weights -> many surviving terms -> many tmp-pool tags;
    # shallower pools keep the SBUF footprint bounded (slower but correct)
    if sum(len(t) for t in terms) > 8:
        bufs = min(bufs, 2)
    import concourse.bacc as bacc
    import concourse.tile as tile
    from concourse import mybir

    f32 = mybir.dt.float32
    Sin = mybir.ActivationFunctionType.Sin
    mult = mybir.AluOpType.mult
    add = mybir.AluOpType.add
    PI = float(np.pi)
    HALF_PI = float(np.pi / 2)
    nb = NPP // nchunk

    cos_ch = sorted({v for tl in terms for _, fs in tl for v, k in fs if k == "c"})
    sin_ch = sorted({v for tl in terms for _, fs in tl for v, k in fs if k == "s"})

    nc = bacc.Bacc("TRN2", target_bir_lowering=False, debug=False,
                   num_devices=N_CORES)
    x_d = nc.dram_tensor("x", [S_CORE, N_QUBITS], f32, kind="ExternalInput").ap()
    o_d = nc.dram_tensor("out", [S_CORE, N_QUBITS], f32,
                         kind="ExternalOutput").ap()
    # chunk k <-> contiguous DRAM block k; within a block, partition-major.
    # Fully linear DMAs; in/out use the same sample mapping so the kernel
    # stays elementwise-consistent.
    x2 = x_d.rearrange("(k p n) c -> k p (n c)", k=nchunk, p=P)
    o2 = o_d.rearrange("(k p n) c -> k p (n c)", k=nchunk, p=P)
    bal = _Balancer(nc, use_gpsimd, gp_two_in)

    with tile.TileContext(nc) as tc:
        with tc.tile_pool(name="xp", bufs=bufs) as xp, \
             tc.tile_pool(name="trig", bufs=bufs) as trigp, \
             tc.tile_pool(name="tmp", bufs=2 * bufs) as tmpp, \
             tc.tile_pool(name="op", bufs=bufs) as op:
            for k in range(nchunk * reps):
                k = k % nchunk
                xt = xp.tile([P, 4 * nb], f32)
                nc.sync.dma_start(xt[:], x2[k])
                xr = xt[:].rearrange("p (n c) -> p n c", c=4)

                feat = {}
                for kind, chans, shift in (("c", cos_ch, HALF_PI),
                                           ("s", sin_ch, 0.0)):
                    if not chans:
                        continue
                    off, st, cnt = _progression(chans)
                    wt = tmpp.tile([P, cnt * nb], f32, tag=f"w{kind}")
                    wr = wt[:].rearrange("p (n c) -> p n c", c=cnt)
                    src = xr[:, :, off:off + st * cnt:st] if cnt > 1 \
                        else xr[:, :, off]
                    dst = wr[:, :, :] if cnt > 1 else wt[:]
                    bal.dve_only(cnt * nb).add_range_wrap(
                        dst, src, shift=shift, bound=PI, period=2 * PI)
                    tt = trigp.tile([P, cnt * nb], f32, tag=f"t{kind}")
                    nc.scalar.activation(tt[:], wt[:], Sin)
                    trr = tt[:].rearrange("p (n c) -> p n c", c=cnt)
                    for v in chans:
                        feat[(v, kind)] = trr[:, :, (v - off) // st]

                ot = op.tile([P, 4 * nb], f32)
                orr = ot[:].rearrange("p (n c) -> p n c", c=4)

                for w in range(N_QUBITS):
                    tl = sorted(terms[w], key=lambda t: -len(t[1]))
                    out_ap = orr[:, :, w]
                    if not tl:
                        nc.vector.memset(out_ap, 0.0)
                        continue
                    # put one degree-1 term last so it fuses into the
                    # final accumulate as a scalar_tensor_tensor
                    for i in range(len(tl) - 1, -1, -1):
                        if len(tl[i][1]) == 1:
                            tl.append(tl.pop(i))
                            break

                    def emit_product(coeff, fs, dst):
                        """dst = coeff * prod(features)"""
                        aps = [feat[f] for f in fs]
                        if len(aps) == 1:
                            bal.one_in(nb).tensor_scalar(dst, aps[0], coeff,
                                                         None, mult)
                            return
                        if len(aps) == 2:
                            bal.two_in(nb).scalar_tensor_tensor(
                                dst, aps[0], coeff, aps[1], mult, mult)
                            return
                        t = tmpp.tile([P, nb], f32, tag="pp")
                        bal.two_in(nb).scalar_tensor_tensor(
                            t[:], aps[0], coeff, aps[1], mult, mult)
                        for ap_ in aps[2:-1]:
                            t2 = tmpp.tile([P, nb], f32, tag="pp2")
                            bal.two_in(nb, is_tt=True).tensor_tensor(t2[:], t[:], ap_, mult)
                            t = t2
                        bal.two_in(nb, is_tt=True).tensor_tensor(dst, t[:], aps[-1], mult)

                    if len(tl) == 1:
                        coeff, fs = tl[0]
                        if fs:
                            emit_product(coeff, fs, out_ap)
                        else:
                            nc.vector.memset(out_ap, coeff)
                        continue

                    acc = None
                    const_c = 0.0
                    for coeff, fs in tl[:-1]:
                        if not fs:
                            const_c += coeff
                            continue
                        t = tmpp.tile([P, nb], f32, tag=f"acc{w}")
                        emit_product(coeff, fs, t[:])
                        if acc is None:
                            acc = t
                        else:
                            t2 = tmpp.tile([P, nb], f32, tag=f"acc{w}b")
                            bal.two_in(nb, is_tt=True).tensor_tensor(t2[:], acc[:], t[:],
                                                         add)
                            acc = t2
                    coeff, fs = tl[-1]
                    final_dst = out_ap
                    if const_c != 0.0:
                        final_dst_t = tmpp.tile([P, nb], f32, tag=f"fc{w}")
                        final_dst = final_dst_t[:]
                    if acc is None:
                        emit_product(coeff, fs, final_dst)
                    elif len(fs) == 1:
                        bal.two_in(nb).scalar_tensor_tensor(
                            final_dst, feat[fs[0]], coeff, acc[:], mult, add)
                    else:
                        t = tmpp.tile([P, nb], f32, tag=f"lt{w}")
                        emit_product(coeff, fs, t[:])
                        bal.two_in(nb, is_tt=True).tensor_tensor(final_dst, acc[:], t[:],
                                                     add)
                    if const_c != 0.0:
                        bal.one_in(nb).tensor_scalar(out_ap, final_dst,
                                                     const_c, None, add)

                getattr(nc, out_eng).dma_start(o2[k], ot[:])

    nc.compile()
    from concourse.bass_interp import get_hw_module
    nc.m = get_hw_module(nc.m)
    return nc


_CACHE = {}


def _get_program(weights):
    key = np.asarray(weights, dtype=np.float64).tobytes()
    if key not in _CACHE:
        C = _solve_C(np.asarray(weights, dtype=np.float64))
        terms = _select_terms(C)
        _CACHE[key] = _build_program(terms)
    return _CACHE[key]


def kernel(x, weights):
    from concourse import bass_utils

    x = np.asarray(x, dtype=np.float32)
    weights = np.asarray(weights, dtype=np.float32)
    assert x.shape == (B_TOTAL, N_QUBITS), x.shape

    nc = _get_program(weights)
    in_maps = [
        {"x": np.ascontiguousarray(x[c * S_CORE:(c + 1) * S_CORE])}
        for c in range(N_CORES)
    ]
    res = bass_utils.run_bass_kernel_spmd(nc, in_maps,
                                          core_ids=list(range(N_CORES)))
    out = np.concatenate([res.results[c]["out"] for c in range(N_CORES)],
                         axis=0)
    return out.astype(np.float32, copy=False)

